# revision 29
# baseline (speedup 1.0000x reference)
"""Block-diagonal linear (grouped GEMM) on 8 TRN2 NeuronCores.

out[b, g*512+n] = sum_k x[b, g*512+k] * blocks[g, k, n]

Sharding: group-parallel — core g computes block g's GEMM.

Input HBM layout is host-packed so all 4 k-tiles of a chunk form one
contiguous-per-partition segment: each chunk is a single 128-descriptor
DMA. HWDGE descriptor generation (~20ns/desc, ~50 desc/us/ring) gates the
pipeline head, so 4x fewer descriptors pulls dense compute several us
earlier. The output keeps per-n-tile DMAs with short (2KB/partition)
descriptors — big-descriptor batched output DMAs measurably depress the
sustained engine clocks — except the final small chunk, whose flush is one
packed 128-descriptor DMA after all compute is done. The packed output is
unpacked on the host.

bf16 everywhere: matmul runs 1 col/cycle at the full 2.4 GHz PE clock
(512-col cadence ~213ns measured) while halving HBM traffic vs fp32;
fp32 PSUM accumulation keeps rel err ~4e-3, inside the 2e-2 gate.
PE floor is 131072 cols / 2.4 GHz = 54.6us/core.

Per-core kernel: out.T = W.T @ x.T as 64 PSUM accumulation groups:
psum[n-tile 128, m 512] += W[k-tile, n-tile].T @ x[k-tile, m-chunk].
Warm-up matmuls on a zeroed tile ramp the PE p-state while the first
DMAs are in flight; PSUM drains are split across DVE and ACT.
"""
import numpy as np
import ml_dtypes

import concourse.bacc as bacc
import concourse.tile as tile
from concourse import mybir
from concourse.bass_utils import run_bass_kernel_spmd

TOKENS = 8192
G = 8
M = 512  # per-block in-features
N = 512  # per-block out-features
P = 128
KT = M // P  # 4 contraction tiles
NT = N // P  # 4 output feature tiles
SUB = 512    # tokens per PSUM group (moving-dim max)
F32 = mybir.dt.float32
BF16 = mybir.dt.bfloat16
NP_BF16 = ml_dtypes.bfloat16

# one 128-descriptor packed DMA per chunk, alternating HWDGE rings (W first
# on sync, chunk 0 parallel on scalar). Chunk 0 is tiny (0.25MB) so it lands
# early even when the scalar ring starts late; chunk 1 (512) rides the sync
# ring right behind W; the tail chunk is tiny so the final flush is short.
CSZ = 1024
CHUNKS = [256, 512] + [CSZ] * 7 + [256]
assert sum(CHUNKS) == TOKENS

_CACHE: dict = {}


def _body(tc, nc, wP, xP, outP):
    with (
        tc.tile_pool(name="wp", bufs=1) as wp,
        tc.tile_pool(name="xin", bufs=8) as xin,
        tc.tile_pool(name="outp", bufs=2) as outp,
        tc.tile_pool(name="pso", bufs=8, space="PSUM") as pso,
    ):
        # W [p, kt, n]: one 128-descriptor DMA, first on the sync ring
        w_t = wp.tile([P, KT, N], BF16, tag="wf")
        nc.sync.dma_start(w_t[:], wP)

        # Warm-up: junk matmuls on a zeroed tile ramp the PE p-state while
        # the first DMAs are still in flight (no DMA dependency).
        warm = wp.tile([P, SUB], BF16, tag="warm")
        nc.vector.memset(warm[:], 0.0)
        ps_w = pso.tile([P, SUB], F32, tag="pso")
        for _ in range(10):
            nc.tensor.matmul(ps_w[:], warm[:, :P], warm[:], start=True, stop=True)

        # cast engines per n-tile: DVE x2 + ACT x2 (GPSIMD cannot read PSUM)
        def cast_v(dst, src):
            nc.vector.tensor_copy(dst, src)

        def cast_s(dst, src):
            nc.scalar.copy(dst, src)

        cast_eng = [cast_v, cast_s, cast_v, cast_s]

        nch = len(CHUNKS)
        x_off = 0
        o_off = 0
        for ci, c in enumerate(CHUNKS):
            # one packed 128-descriptor input DMA per chunk; even chunks ride
            # the scalar ring, odd chunks follow W on the sync ring. The two
            # tail chunks get their own persistent exact-shape tiles.
            if c == CSZ:
                x_t = xin.tile([P, KT, CSZ], BF16, tag="x")
            else:
                x_t = wp.tile([P, KT, c], BF16, tag=f"xtail{ci}")
            eng = nc.scalar if ci % 2 == 0 else nc.sync
            eng.dma_start(x_t[:], xP[:, x_off:x_off + KT * c])

            last = ci == nch - 1
            if last:
                # single tile so the final flush is packed 128-desc DMAs
                ot_l = wp.tile([P, NT, c], BF16, tag="otlast")
                ots = None
            else:
                ots = [outp.tile([P, CSZ], BF16, tag=f"o{nt}", name=f"ot{nt}")
                       for nt in range(NT)]
            # the last chunk runs as two 256-token groups so its casts
            # overlap its own matmuls and the flush can start sooner
            sub = 256 if last else SUB
            for s0 in range(0, c, sub):
                sw = min(sub, c - s0)
                for nt in range(NT):
                    ps_o = pso.tile([P, SUB], F32, tag="pso")
                    for j in range(KT):
                        nc.tensor.matmul(
                            ps_o[:, :sw],
                            w_t[:, j, nt * P:(nt + 1) * P],
                            x_t[:, j, s0:s0 + sw],
                            start=(j == 0),
                            stop=(j == KT - 1),
                        )
                    if last:
                        cast_eng[nt](ot_l[:, nt, s0:s0 + sw], ps_o[:, :sw])
                    else:
                        cast_eng[nt](ots[nt][:, s0:s0 + sw], ps_o[:, :sw])
            # flush: per-n-tile SWDGE DMAs (short descriptors) into the packed
            # output; the last chunk splits into two packed halves on the two
            # idle HWDGE rings so descriptor gen starts after the nt0/nt1
            # casts and both halves generate in parallel
            if last:
                nc.sync.dma_start(outP[:, o_off:o_off + 2 * c], ot_l[:, 0:2, :])
                nc.scalar.dma_start(
                    outP[:, o_off + 2 * c:o_off + NT * c], ot_l[:, 2:4, :]
                )
            else:
                for nt in range(NT):
                    nc.gpsimd.dma_start(
                        outP[:, o_off + nt * c:o_off + (nt + 1) * c],
                        ots[nt][:, :c],
                    )
            x_off += KT * c
            o_off += NT * c


def _build():
    nc = bacc.Bacc("TRN2", target_bir_lowering=False, debug=False, num_devices=G)
    wP = nc.dram_tensor("wP", [P, KT * N], BF16, kind="ExternalInput").ap()
    xP = nc.dram_tensor("xP", [P, KT * TOKENS], BF16, kind="ExternalInput").ap()
    outP = nc.dram_tensor("outP", [P, NT * TOKENS], BF16, kind="ExternalOutput").ap()
    with tile.TileContext(nc) as tc:
        _body(tc, nc, wP, xP, outP)
    nc.compile()
    return nc


def _run(in_maps, **kwargs):
    if "nc" not in _CACHE:
        _CACHE["nc"] = _build()
    return run_bass_kernel_spmd(_CACHE["nc"], in_maps, list(range(G)), **kwargs)


def _in_maps(x, blocks):
    maps = []
    for g in range(G):
        xTg = np.ascontiguousarray(x[:, g * M:(g + 1) * M].T).astype(NP_BF16)
        w = blocks[g].astype(NP_BF16)
        wpk = w.reshape(KT, P, N).transpose(1, 0, 2).reshape(P, KT * N)
        segs = []
        m0 = 0
        for c in CHUNKS:
            blk = xTg[:, m0:m0 + c].reshape(KT, P, c).transpose(1, 0, 2)
            segs.append(blk.reshape(P, KT * c))
            m0 += c
        maps.append({
            "wP": np.ascontiguousarray(wpk),
            "xP": np.ascontiguousarray(np.concatenate(segs, axis=1)),
        })
    return maps


def kernel(x, blocks):
    x = np.asarray(x, dtype=np.float32)
    blocks = np.asarray(blocks, dtype=np.float32)
    res = _run(_in_maps(x, blocks))
    out = np.empty((TOKENS, G * N), dtype=np.float32)
    for g in range(G):
        oP = res.results[g]["outP"]  # [P, NT*TOKENS] bf16, chunk-packed
        off = m0 = 0
        for c in CHUNKS:
            blk = oP[:, off:off + NT * c].reshape(P, NT, c)
            out[m0:m0 + c, g * N:(g + 1) * N] = (
                blk.transpose(2, 1, 0).reshape(c, NT * P).astype(np.float32)
            )
            off += NT * c
            m0 += c
    return out


# revision 30
# speedup vs baseline: 1.0149x; 1.0149x over previous
"""Block-diagonal linear (grouped GEMM) on 8 TRN2 NeuronCores.

out[b, g*512+n] = sum_k x[b, g*512+k] * blocks[g, k, n]

Sharding: group-parallel — core g computes block g's GEMM.

Input HBM layout is host-packed so all 4 k-tiles of a chunk form one
contiguous-per-partition segment: each chunk is a single 128-descriptor
DMA. HWDGE descriptor generation (~20ns/desc, ~50 desc/us/ring) gates the
pipeline head, so 4x fewer descriptors pulls dense compute several us
earlier. The output keeps per-n-tile DMAs with short (2KB/partition)
descriptors — big-descriptor batched output DMAs measurably depress the
sustained engine clocks — except the final small chunk, whose flush is one
packed 128-descriptor DMA after all compute is done. The packed output is
unpacked on the host.

bf16 everywhere: matmul runs 1 col/cycle at the full 2.4 GHz PE clock
(512-col cadence ~213ns measured) while halving HBM traffic vs fp32;
fp32 PSUM accumulation keeps rel err ~4e-3, inside the 2e-2 gate.
PE floor is 131072 cols / 2.4 GHz = 54.6us/core.

Per-core kernel: out.T = W.T @ x.T as 64 PSUM accumulation groups:
psum[n-tile 128, m 512] += W[k-tile, n-tile].T @ x[k-tile, m-chunk].
Warm-up matmuls on a zeroed tile ramp the PE p-state while the first
DMAs are in flight; PSUM drains are split across DVE and ACT.
"""
import numpy as np
import ml_dtypes

import concourse.bacc as bacc
import concourse.tile as tile
from concourse import mybir
from concourse.bass_utils import run_bass_kernel_spmd

TOKENS = 8192
G = 8
M = 512  # per-block in-features
N = 512  # per-block out-features
P = 128
KT = M // P  # 4 contraction tiles
NT = N // P  # 4 output feature tiles
SUB = 512    # tokens per PSUM group (moving-dim max)
F32 = mybir.dt.float32
BF16 = mybir.dt.bfloat16
NP_BF16 = ml_dtypes.bfloat16

# one 128-descriptor packed DMA per chunk, alternating HWDGE rings (W first
# on sync, chunk 0 parallel on scalar). Chunk 0 is tiny (0.25MB) so it lands
# early even when the scalar ring starts late; chunk 1 (512) rides the sync
# ring right behind W; the tail chunk is tiny so the final flush is short.
CSZ = 1024
CHUNKS = [256, 512] + [CSZ] * 7 + [256]
assert sum(CHUNKS) == TOKENS

_CACHE: dict = {}


def _body(tc, nc, wP, xP, outP):
    with (
        tc.tile_pool(name="wp", bufs=1) as wp,
        tc.tile_pool(name="xin", bufs=8) as xin,
        tc.tile_pool(name="outp", bufs=3) as outp,
        tc.tile_pool(name="pso", bufs=8, space="PSUM") as pso,
    ):
        # W [p, kt, n]: one 128-descriptor DMA, first on the sync ring
        w_t = wp.tile([P, KT, N], BF16, tag="wf")
        nc.sync.dma_start(w_t[:], wP)

        # Warm-up: junk matmuls on a zeroed tile ramp the PE p-state while
        # the first DMAs are still in flight (no DMA dependency).
        warm = wp.tile([P, SUB], BF16, tag="warm")
        nc.vector.memset(warm[:], 0.0)
        ps_w = pso.tile([P, SUB], F32, tag="pso")
        for _ in range(10):
            nc.tensor.matmul(ps_w[:], warm[:, :P], warm[:], start=True, stop=True)

        # cast engines per n-tile: DVE x2 + ACT x2 (GPSIMD cannot read PSUM)
        def cast_v(dst, src):
            nc.vector.tensor_copy(dst, src)

        def cast_s(dst, src):
            nc.scalar.copy(dst, src)

        cast_eng = [cast_v, cast_s, cast_v, cast_s]

        nch = len(CHUNKS)
        x_off = 0
        o_off = 0
        for ci, c in enumerate(CHUNKS):
            # one packed 128-descriptor input DMA per chunk; even chunks ride
            # the scalar ring, odd chunks follow W on the sync ring. The two
            # tail chunks get their own persistent exact-shape tiles.
            if c == CSZ:
                x_t = xin.tile([P, KT, CSZ], BF16, tag="x")
            else:
                x_t = wp.tile([P, KT, c], BF16, tag=f"xtail{ci}")
            eng = nc.scalar if ci % 2 == 0 else nc.sync
            eng.dma_start(x_t[:], xP[:, x_off:x_off + KT * c])

            last = ci == nch - 1
            if last:
                # single tile so the final flush is packed 128-desc DMAs
                ot_l = wp.tile([P, NT, c], BF16, tag="otlast")
                ots = None
            else:
                ots = [outp.tile([P, CSZ], BF16, tag=f"o{nt}", name=f"ot{nt}")
                       for nt in range(NT)]
            # the last chunk runs as two 256-token groups so its casts
            # overlap its own matmuls and the flush can start sooner
            sub = 256 if last else SUB
            for s0 in range(0, c, sub):
                sw = min(sub, c - s0)
                for nt in range(NT):
                    ps_o = pso.tile([P, SUB], F32, tag="pso")
                    for j in range(KT):
                        nc.tensor.matmul(
                            ps_o[:, :sw],
                            w_t[:, j, nt * P:(nt + 1) * P],
                            x_t[:, j, s0:s0 + sw],
                            start=(j == 0),
                            stop=(j == KT - 1),
                        )
                    if last:
                        cast_eng[nt](ot_l[:, nt, s0:s0 + sw], ps_o[:, :sw])
                    else:
                        cast_eng[nt](ots[nt][:, s0:s0 + sw], ps_o[:, :sw])
            # flush: per-n-tile SWDGE DMAs (short descriptors) into the packed
            # output; the last chunk splits into two packed halves on the two
            # idle HWDGE rings so descriptor gen starts after the nt0/nt1
            # casts and both halves generate in parallel
            if last:
                nc.sync.dma_start(outP[:, o_off:o_off + 2 * c], ot_l[:, 0:2, :])
                nc.scalar.dma_start(
                    outP[:, o_off + 2 * c:o_off + NT * c], ot_l[:, 2:4, :]
                )
            else:
                for nt in range(NT):
                    nc.gpsimd.dma_start(
                        outP[:, o_off + nt * c:o_off + (nt + 1) * c],
                        ots[nt][:, :c],
                    )
            x_off += KT * c
            o_off += NT * c


def _build():
    nc = bacc.Bacc("TRN2", target_bir_lowering=False, debug=False, num_devices=G)
    wP = nc.dram_tensor("wP", [P, KT * N], BF16, kind="ExternalInput").ap()
    xP = nc.dram_tensor("xP", [P, KT * TOKENS], BF16, kind="ExternalInput").ap()
    outP = nc.dram_tensor("outP", [P, NT * TOKENS], BF16, kind="ExternalOutput").ap()
    with tile.TileContext(nc) as tc:
        _body(tc, nc, wP, xP, outP)
    nc.compile()
    return nc


def _run(in_maps, **kwargs):
    if "nc" not in _CACHE:
        _CACHE["nc"] = _build()
    return run_bass_kernel_spmd(_CACHE["nc"], in_maps, list(range(G)), **kwargs)


def _in_maps(x, blocks):
    maps = []
    for g in range(G):
        xTg = np.ascontiguousarray(x[:, g * M:(g + 1) * M].T).astype(NP_BF16)
        w = blocks[g].astype(NP_BF16)
        wpk = w.reshape(KT, P, N).transpose(1, 0, 2).reshape(P, KT * N)
        segs = []
        m0 = 0
        for c in CHUNKS:
            blk = xTg[:, m0:m0 + c].reshape(KT, P, c).transpose(1, 0, 2)
            segs.append(blk.reshape(P, KT * c))
            m0 += c
        maps.append({
            "wP": np.ascontiguousarray(wpk),
            "xP": np.ascontiguousarray(np.concatenate(segs, axis=1)),
        })
    return maps


def kernel(x, blocks):
    x = np.asarray(x, dtype=np.float32)
    blocks = np.asarray(blocks, dtype=np.float32)
    res = _run(_in_maps(x, blocks))
    out = np.empty((TOKENS, G * N), dtype=np.float32)
    for g in range(G):
        oP = res.results[g]["outP"]  # [P, NT*TOKENS] bf16, chunk-packed
        off = m0 = 0
        for c in CHUNKS:
            blk = oP[:, off:off + NT * c].reshape(P, NT, c)
            out[m0:m0 + c, g * N:(g + 1) * N] = (
                blk.transpose(2, 1, 0).reshape(c, NT * P).astype(np.float32)
            )
            off += NT * c
            m0 += c
    return out
